# revision 14
# baseline (speedup 1.0000x reference)
"""GQA attention block (RMSNorm-QK, causal, GQA) on 8 trn2 NeuronCores — v2.

Sharding: batch over groups of 4 cores; stride-4 query interleave within a
batch. Core c handles batch c//4 and query tokens {j, j+4, ..., j+2044}
(j = c%4), so the causal structure is IDENTICAL on every core: for key tile
kt (128 keys), query columns < 32*kt are fully masked (skipped entirely),
columns [32kt, 32kt+32) are diagonal (one shared [128,32] 0/1 mask), and
the rest are fully valid. Scores / exp / denominator / AV all run on the
causally-valid suffix [32kt, 512) only — ~47% less attention work than the
full rectangle, with zero collectives and one uniform SPMD program.

All matmuls are bf16 (1 cycle/row at any free size on the PE; error budget
2e-2 >> bf16's ~1e-3). Activations are feature-major ("T layout"); V is
projected token-major directly (x-tiles stationary), so the kernel needs no
transposes at all. Partition-dim reductions (RMS sum-of-squares, softmax
denominators) and per-token broadcasts are rank-1 matmuls; reciprocals use
the fast custom-DVE op (~18 bits, ~5x faster than nc.vector.reciprocal).
Score tiles for key-tile pairs (p, 16-p) pack into a single PSUM bank so
exp runs as one activation per bank. Softmax needs no max subtraction:
RMS-normalized q,k bound |scores|/sqrt(D) <= sqrt(D).
"""

import math
import numpy as np
from contextlib import ExitStack

import ml_dtypes
import concourse.bass as bass
import concourse.mybir as mybir
import concourse.tile as tile
from concourse import bacc
from concourse.bass_utils import run_bass_kernel_spmd

F32 = mybir.dt.float32
F32R = mybir.dt.float32r
BF16 = mybir.dt.bfloat16
ADD = mybir.AluOpType.add
MULT = mybir.AluOpType.mult
EXP = mybir.ActivationFunctionType.Exp
SQRT = mybir.ActivationFunctionType.Sqrt
SQUARE = mybir.ActivationFunctionType.Square

BF = ml_dtypes.bfloat16
EPS = 1e-8


def full_cfg():
    return dict(B=2, S=2048, E=2048, D=128, G=2)


def derived(cfg):
    B, S, E, D, G = cfg["B"], cfg["S"], cfg["E"], cfg["D"], cfg["G"]
    NH = E // D            # 16 query heads == E blocks of 128
    ET = E // 128          # 16 contraction tiles of E
    NKT = S // 128         # 16 key tiles
    QPC = S // 4           # 512 queries per core (stride-4 stripe)
    GS = NH // G           # 8 heads per kv group
    assert D == 128 and QPC == 512
    return NH, ET, NKT, QPC, GS


# Phase-3 tiling (per head PAIR sharing a kv group): key tiles kt<8 are
# per-head, two per 2-bank mega at bank-aligned offsets; key tiles kt>=8
# (suffix width w<=256) merge BOTH heads into one score matmul with a
# 2-head-interleaved output (span 2w <= one PSUM bank), halving their
# score-matmul count. (kt, col-offset) per mega:
A_GROUPS = [(0, 1), (2, 3), (4, 5), (6, 7)]
M_GROUPS = [
    [(8, 0), (9, 512), (15, 960)],
    [(10, 0), (14, 384), (11, 512), (13, 832)],
    [(12, 0)],
]
M_USED = [1024, 1024, 256]


def build_program(cfg):
    B, S, E, D, G = cfg["B"], cfg["S"], cfg["E"], cfg["D"], cfg["G"]
    NH, ET, NKT, QPC, GS = derived(cfg)
    SCALE = 1.0 / math.sqrt(D)
    KC = 512
    NKC = S // KC

    nc = bacc.Bacc()
    xT_d = nc.dram_tensor("xT", [E, S], BF16, kind="ExternalInput")
    xq_d = nc.dram_tensor("xq", [E, QPC], BF16, kind="ExternalInput")
    wq_d = nc.dram_tensor("WqP", [128, NH, ET * 128], BF16, kind="ExternalInput")
    wo_d = nc.dram_tensor("WoP", [128, ET, ET * 128], BF16, kind="ExternalInput")
    wk_d = nc.dram_tensor("WkP", [128, ET * G * 128], BF16, kind="ExternalInput")
    wv_d = nc.dram_tensor("WvP", [128, ET * G * 128], BF16, kind="ExternalInput")
    bq_d = nc.dram_tensor("bq_t", [128, NH], F32, kind="ExternalInput")
    bk_d = nc.dram_tensor("bk_t", [128, G], F32, kind="ExternalInput")
    bv_d = nc.dram_tensor("bv_r", [1, G * 128], BF16, kind="ExternalInput")
    bo_d = nc.dram_tensor("bo_t", [128, ET], F32, kind="ExternalInput")
    gq_d = nc.dram_tensor("gq_r", [1, 128], BF16, kind="ExternalInput")
    gk_d = nc.dram_tensor("gk_r", [1, 128], BF16, kind="ExternalInput")
    dm_d = nc.dram_tensor("dmask", [128, 32], BF16, kind="ExternalInput")
    dm2_d = nc.dram_tensor("dmask2", [128, 64], BF16, kind="ExternalInput")
    out_d = nc.dram_tensor("outT", [E, QPC], F32, kind="ExternalOutput")

    def r(ap):
        return ap if ap.dtype == F32R else ap.bitcast(F32R)

    xT_r = xT_d.rearrange("(t p) s -> p t s", p=128)    # [128, ET, S]
    xq_r = xq_d.rearrange("(t p) q -> p t q", p=128)    # [128, ET, QPC]

    with tile.TileContext(nc) as tc, ExitStack() as top:
        consts = top.enter_context(tc.tile_pool(name="consts", bufs=1))
        persist = top.enter_context(tc.tile_pool(name="persist", bufs=1))
        xqp = top.enter_context(tc.tile_pool(name="xqp", bufs=1))
        wqp = top.enter_context(tc.tile_pool(name="wqs", bufs=6))

        ktn = [persist.tile([128, S], BF16, tag=f"ktn{g}", name=f"ktn{g}")
               for g in range(G)]
        vtok = persist.tile([128, NKT, G * 128], BF16, tag="vtok")
        qtn = persist.tile([128, NH, QPC], BF16, tag="qtn")
        ctxt = persist.tile([128, ET, QPC], BF16, tag="ctxt")

        # ------------- phase 1: K/V projection over all tokens ------------
        with ExitStack() as p1:
            xsp = p1.enter_context(tc.tile_pool(name="xs", bufs=4))
            wkvp = p1.enter_context(tc.tile_pool(name="wkv", bufs=1))
            tmp = p1.enter_context(tc.tile_pool(name="tmp1", bufs=3))

            # startup DMA order: tiny "starter" transfers first so the
            # first matmuls are gated on ~0.4MB, then the bulk, then
            # prefetches, then consts (needed ~20us in).
            wk_sb = wkvp.tile([128, ET * G * 128], BF16, tag="wk")
            nc.sync.dma_start(out=wk_sb[:, 0:512], in_=wk_d[:, 0:512])
            xts = []
            xt0 = xsp.tile([128, ET, KC], BF16, tag="xt", name="xt0")
            nc.sync.dma_start(out=xt0[:, 0:2, :], in_=xT_r[:, 0:2, 0:KC])
            nc.sync.dma_start(out=wk_sb[:, 512:2048], in_=wk_d[:, 512:2048])
            nc.sync.dma_start(out=xt0[:, 2:8, :], in_=xT_r[:, 2:8, 0:KC])
            nc.sync.dma_start(out=wk_sb[:, 2048:4096],
                              in_=wk_d[:, 2048:4096])
            nc.sync.dma_start(out=xt0[:, 8:16, :], in_=xT_r[:, 8:16, 0:KC])
            xts.append(xt0)
            wv_sb = wkvp.tile([128, ET * G * 128], BF16, tag="wv")
            for i in range(2):
                nc.sync.dma_start(out=wv_sb[:, i * 2048:(i + 1) * 2048],
                                  in_=wv_d[:, i * 2048:(i + 1) * 2048])
            xt1 = xsp.tile([128, ET, KC], BF16, tag="xt", name="xt1")
            nc.sync.dma_start(out=xt1, in_=xT_r[:, :, KC:2 * KC])
            xts.append(xt1)
            xq_sb = xqp.tile([128, ET, QPC], BF16, tag="xq")
            nc.sync.dma_start(out=xq_sb, in_=xq_r)

            ones_col_bf = consts.tile([128, 1], BF16)
            nc.vector.memset(ones_col_bf, 1.0)
            ones_row_bf = consts.tile([1, 128], BF16)
            nc.vector.memset(ones_row_bf, 1.0)
            eps_t = consts.tile([1, 1], F32)
            nc.vector.memset(eps_t, EPS)
            gq_sb = consts.tile([1, 128], BF16)
            nc.sync.dma_start(out=gq_sb, in_=gq_d[:, :])
            gk_sb = consts.tile([1, 128], BF16)
            nc.sync.dma_start(out=gk_sb, in_=gk_d[:, :])
            bq_sb = consts.tile([128, NH], F32)
            nc.sync.dma_start(out=bq_sb, in_=bq_d[:, :])
            bk_sb = consts.tile([128, G], F32)
            nc.sync.dma_start(out=bk_sb, in_=bk_d[:, :])
            bv_sb = consts.tile([1, G * 128], BF16)
            nc.sync.dma_start(out=bv_sb, in_=bv_d[:, :])
            bo_sb = consts.tile([128, ET], F32)
            nc.sync.dma_start(out=bo_sb, in_=bo_d[:, :])
            dmask = consts.tile([128, 32], BF16)
            nc.sync.dma_start(out=dmask, in_=dm_d[:, :])
            dmask2 = consts.tile([128, 64], BF16)
            nc.sync.dma_start(out=dmask2, in_=dm2_d[:, :])
            wq_tiles = []
            for qc in range(2):
                wq_sb = wqp.tile([128, ET * 128], BF16, tag="wq", name="wq")
                nc.sync.dma_start(out=wq_sb, in_=wq_d[:, qc, :])
                wq_tiles.append(wq_sb)
            pk = p1.enter_context(tc.tile_pool(name="pk", bufs=1, space="PSUM"))
            pv = p1.enter_context(tc.tile_pool(name="pv", bufs=1, space="PSUM"))
            pssq = p1.enter_context(tc.tile_pool(name="pssq", bufs=2, space="PSUM"))
            pbc = p1.enter_context(tc.tile_pool(name="pbc", bufs=2, space="PSUM"))

            q_top, q_mid, q_bot = [], [], []

            def pop(q):
                if q:
                    q.pop(0)()

            for kc in range(NKC):
                if kc + 2 < NKC:
                    xt = xsp.tile([128, ET, KC], BF16, tag="xt", name="xt")
                    nc.sync.dma_start(
                        out=xt, in_=xT_r[:, :, (kc + 2) * KC:(kc + 3) * KC])
                    xts.append(xt)
                xc = xts[kc]
                pop(q_top)
                acck = pk.tile([128, G, KC], F32, tag="acck", name="acck")
                for g in range(G):
                    for et in range(ET):
                        nc.tensor.matmul(
                            acck[:, g, :],
                            lhsT=wk_sb[:, et * 256 + g * 128:
                                       et * 256 + (g + 1) * 128],
                            rhs=xc[:, et, :], start=(et == 0),
                            stop=(et == ET - 1))
                pop(q_mid)
                accv = pv.tile([128, 4, G * 128], F32, tag="accv", name="accv")
                for s in range(4):
                    for et in range(ET):
                        nc.tensor.matmul(
                            accv[:, s, :],
                            lhsT=xc[:, et, s * 128:(s + 1) * 128],
                            rhs=wv_sb[:, et * 256:(et + 1) * 256],
                            start=(et == 0), stop=False)
                    nc.tensor.matmul(accv[:, s, :], lhsT=ones_row_bf,
                                     rhs=bv_sb, start=False, stop=True)
                pop(q_bot)

                def top_f(kc=kc, acck=acck, accv=accv):
                    outs = []
                    for g in range(G):
                        sq = tmp.tile([128, KC], BF16, tag="sq", name="sq",
                                      bufs=3)
                        nc.scalar.activation(out=sq, in_=acck[:, g, :],
                                             func=SQUARE,
                                             bias=bk_sb[:, g:g + 1])
                        vb = tmp.tile([128, KC], F32, tag="vb", name="vb",
                                      bufs=5)
                        nc.vector.tensor_scalar(
                            out=vb, in0=acck[:, g, :],
                            scalar1=bk_sb[:, g:g + 1], scalar2=None, op0=ADD)
                        outs.append((sq, vb))
                    for s in range(4):
                        nc.scalar.copy(out=vtok[:, kc * 4 + s, :],
                                       in_=accv[:, s, :])
                    q_mid.append(lambda kc=kc, outs=outs: mid_f(kc, outs))

                def mid_f(kc, outs):
                    outs2 = []
                    for g in range(G):
                        sq, vb = outs[g]
                        ssq = pssq.tile([1, KC], F32, tag="ssq", name="ssq")
                        nc.tensor.matmul(ssq, lhsT=ones_col_bf, rhs=sq,
                                         start=True, stop=True)
                        rms = tmp.tile([1, KC], F32, tag="rms", name="rms",
                                       bufs=3)
                        nc.scalar.activation(out=rms, in_=ssq, func=SQRT,
                                             scale=1.0 / D, bias=eps_t[:, :])
                        rinv = tmp.tile([1, KC], F32, tag="rinv", name="rinv",
                                        bufs=3)
                        nc.vector.reciprocal_approx_fast(out=rinv, in_=rms)
                        rinv_r = tmp.tile([1, KC], BF16, tag="rinvr",
                                          name="rinvr", bufs=5)
                        nc.vector.tensor_copy(out=rinv_r, in_=rinv)
                        outs2.append((vb, rinv_r))
                    q_bot.append(lambda kc=kc, outs2=outs2: bot_f(kc, outs2))

                def bot_f(kc, outs2):
                    for g in range(G):
                        vb, rinv_r = outs2[g]
                        bc = pbc.tile([128, KC], F32, tag="bc", name="bc")
                        nc.tensor.matmul(bc, lhsT=gk_sb, rhs=rinv_r,
                                         start=True, stop=True)
                        nc.vector.tensor_tensor(
                            out=ktn[g][:, kc * KC:(kc + 1) * KC],
                            in0=vb, in1=bc, op=MULT)

                q_top.append(top_f)
            while q_top or q_mid or q_bot:
                pop(q_top)
                pop(q_mid)
                pop(q_bot)

        # ------------- phase 2: Q projection (own 512 queries) ------------
        with ExitStack() as p2:
            tmp2 = p2.enter_context(tc.tile_pool(name="tmp2", bufs=3))
            pq = p2.enter_context(tc.tile_pool(name="pq", bufs=2, space="PSUM"))
            pssq2 = p2.enter_context(tc.tile_pool(name="pssq2", bufs=2,
                                                  space="PSUM"))
            pbcq = p2.enter_context(tc.tile_pool(name="pbcq", bufs=2,
                                                 space="PSUM"))
            for qc in range(2, 4):
                wq_sb = wqp.tile([128, ET * 128], BF16, tag="wq", name="wq")
                nc.sync.dma_start(out=wq_sb, in_=wq_d[:, qc, :])
                wq_tiles.append(wq_sb)
            q_top, q_mid, q_bot = [], [], []
            for qc in range(NH):
                if qc + 4 < NH:
                    wq_sb = wqp.tile([128, ET * 128], BF16, tag="wq",
                                     name="wq")
                    nc.sync.dma_start(out=wq_sb, in_=wq_d[:, qc + 4, :])
                    wq_tiles.append(wq_sb)
                pop(q_top)
                acc = pq.tile([128, QPC], F32, tag="qacc", name="qacc")
                for et in range(ET):
                    nc.tensor.matmul(
                        acc,
                        lhsT=wq_tiles[qc][:, et * 128:(et + 1) * 128],
                        rhs=xq_sb[:, et, :],
                        start=(et == 0), stop=(et == ET - 1))
                pop(q_mid)
                pop(q_bot)

                def top_f(qc=qc, acc=acc):
                    sq = tmp2.tile([128, QPC], BF16, tag="sq", name="qsq")
                    nc.scalar.activation(out=sq, in_=acc, func=SQUARE,
                                         bias=bq_sb[:, qc:qc + 1])
                    vb = tmp2.tile([128, QPC], F32, tag="vb", name="qvb",
                                   bufs=4)
                    nc.vector.tensor_scalar(
                        out=vb, in0=acc, scalar1=bq_sb[:, qc:qc + 1],
                        scalar2=None, op0=ADD)

                    def mid_f(qc=qc, sq=sq, vb=vb):
                        ssq = pssq2.tile([1, QPC], F32, tag="ssq",
                                         name="qssq")
                        nc.tensor.matmul(ssq, lhsT=ones_col_bf, rhs=sq,
                                         start=True, stop=True)
                        rms = tmp2.tile([1, QPC], F32, tag="rms", name="qrms")
                        nc.scalar.activation(out=rms, in_=ssq, func=SQRT,
                                             scale=1.0 / D, bias=eps_t[:, :])
                        rinv = tmp2.tile([1, QPC], F32, tag="rinv",
                                         name="qrinv", bufs=3)
                        nc.vector.reciprocal_approx_fast(out=rinv, in_=rms)
                        rinv_r = tmp2.tile([1, QPC], BF16, tag="rinvr",
                                           name="qrinvr", bufs=4)
                        nc.vector.tensor_copy(out=rinv_r, in_=rinv)

                        def bot_f(qc=qc, vb=vb, rinv_r=rinv_r):
                            bc = pbcq.tile([128, QPC], F32, tag="bc",
                                           name="qbc")
                            nc.tensor.matmul(bc, lhsT=gq_sb, rhs=rinv_r,
                                             start=True, stop=True)
                            nc.vector.tensor_tensor(out=qtn[:, qc, :],
                                                    in0=vb, in1=bc, op=MULT)
                        q_bot.append(bot_f)
                    q_mid.append(mid_f)
                q_top.append(top_f)
            while q_top or q_mid or q_bot:
                pop(q_top)
                pop(q_mid)
                pop(q_bot)

        # ---------- phase 3: causal attention + phase 4: out proj ---------
        with ExitStack() as p34:
            ptp = p34.enter_context(tc.tile_pool(name="pt", bufs=5))
            tmp3 = p34.enter_context(tc.tile_pool(name="tmp3", bufs=3))
            wop = p34.enter_context(tc.tile_pool(name="wos", bufs=16))
            osb = p34.enter_context(tc.tile_pool(name="osb", bufs=3))
            psc = p34.enter_context(tc.tile_pool(name="psc", bufs=2,
                                                 space="PSUM"))
            pden = p34.enter_context(tc.tile_pool(name="pden", bufs=2,
                                                  space="PSUM"))
            pcx = p34.enter_context(tc.tile_pool(name="pcx", bufs=2,
                                                 space="PSUM"))
            pend_exp = []
            pend_acc = []
            pendH = []

            def post_exp_a(h, g, grp, sc, den, cx):
                ka, kb = grp
                wb = 512 - 32 * kb
                used = 512 + wb
                pt = ptp.tile([128, 2 * QPC], BF16, tag="pt", name="pt")
                nc.scalar.activation(out=pt[:, 0:used], in_=sc[:, 0:used],
                                     func=EXP, scale=SCALE)
                for kt, off in ((ka, 0), (kb, 512)):
                    nc.vector.tensor_tensor(out=pt[:, off:off + 32],
                                            in0=pt[:, off:off + 32],
                                            in1=dmask, op=MULT)

                def post_acc():
                    for kt, off in ((ka, 0), (kb, 512)):
                        w = 512 - 32 * kt
                        nc.tensor.matmul(den[:, 32 * kt:512],
                                         lhsT=ones_col_bf,
                                         rhs=pt[:, off:off + w],
                                         start=(kt == 0), stop=False)
                        nc.tensor.matmul(
                            cx[:, 32 * kt:512],
                            lhsT=vtok[:, kt, g * 128:(g + 1) * 128],
                            rhs=pt[:, off:off + w],
                            start=(kt == 0), stop=False)
                pend_acc.append(post_acc)

            def post_exp_m(h0, g, grp, used, sc, dens, cxs):
                pt = ptp.tile([128, 2 * QPC], BF16, tag="pt", name="pt")
                nc.scalar.activation(out=pt[:, 0:used], in_=sc[:, 0:used],
                                     func=EXP, scale=SCALE)
                for kt, off in grp:
                    nc.vector.tensor_tensor(out=pt[:, off:off + 64],
                                            in0=pt[:, off:off + 64],
                                            in1=dmask2, op=MULT)

                def post_acc():
                    for kt, off in grp:
                        w = 512 - 32 * kt
                        pt_r = pt[:, off:off + 2 * w].rearrange(
                            "p (w two) -> p two w", two=2)
                        for i in range(2):
                            nc.tensor.matmul(dens[i][:, 32 * kt:512],
                                             lhsT=ones_col_bf,
                                             rhs=pt_r[:, i, :],
                                             start=False, stop=(kt == 12))
                            nc.tensor.matmul(
                                cxs[i][:, 32 * kt:512],
                                lhsT=vtok[:, kt, g * 128:(g + 1) * 128],
                                rhs=pt_r[:, i, :],
                                start=False, stop=(kt == 12))
                pend_acc.append(post_acc)

            def post_head(h, den, cx):
                rd = tmp3.tile([1, QPC], F32, tag="rd", name="rd")
                nc.vector.reciprocal_approx_fast(out=rd, in_=den)
                rd_bf = tmp3.tile([1, QPC], BF16, tag="rdbf", name="rdbf")
                nc.vector.tensor_copy(out=rd_bf, in_=rd)
                bc2 = psc.tile([128, 2 * QPC], F32, tag="sc", name="bc2")
                nc.tensor.matmul(bc2[:, 0:QPC], lhsT=ones_row_bf, rhs=rd_bf,
                                 start=True, stop=True)
                bc2s = tmp3.tile([128, QPC], F32, tag="bc2s", name="bc2s")
                nc.scalar.copy(out=bc2s, in_=bc2[:, 0:QPC])
                nc.vector.tensor_tensor(out=ctxt[:, h, :], in0=cx, in1=bc2s,
                                        op=MULT)

            # prefetch all Wo tiles while Sync is otherwise idle in phase 3
            wo_tiles = []
            for c2 in range(ET):
                wo_sb = wop.tile([128, ET * 128], BF16, tag="wo", name="wo")
                nc.sync.dma_start(out=wo_sb, in_=wo_d[:, c2, :])
                wo_tiles.append(wo_sb)

            for hp in range(NH // 2):
                h0 = 2 * hp
                g = h0 // GS
                dens = [pden.tile([1, QPC], F32, tag="den", name="den")
                        for _ in range(2)]
                cxs = [pcx.tile([128, QPC], F32, tag="cx", name="cx")
                       for _ in range(2)]
                tiles = ([("A", 0, grp) for grp in A_GROUPS]
                         + [("A", 1, grp) for grp in A_GROUPS]
                         + [("M", mi, grp) for mi, grp in
                            enumerate(M_GROUPS)])
                for ti, (kind, idx, grp) in enumerate(tiles):
                    if pend_acc:
                        pend_acc.pop(0)()
                    if pend_exp:
                        pend_exp.pop(0)()
                    if ti in (2, 5) and pendH:
                        pendH.pop(0)()
                    sc = psc.tile([128, 2 * QPC], F32, tag="sc", name="sc")
                    if kind == "A":
                        h = h0 + idx
                        for kt, off in ((grp[0], 0), (grp[1], 512)):
                            w = 512 - 32 * kt
                            nc.tensor.matmul(
                                sc[:, off:off + w],
                                lhsT=ktn[g][:, kt * 128:(kt + 1) * 128],
                                rhs=qtn[:, h, 32 * kt:512],
                                start=True, stop=True)
                        pend_exp.append(
                            lambda h=h, g=g, grp=grp, sc=sc,
                            den=dens[idx], cx=cxs[idx]:
                            post_exp_a(h, g, grp, sc, den, cx))
                    else:
                        for kt, off in grp:
                            w = 512 - 32 * kt
                            sc_i = sc[:, off:off + 2 * w].rearrange(
                                "p (w two) -> p two w", two=2)
                            nc.tensor.matmul(
                                sc_i,
                                lhsT=ktn[g][:, kt * 128:(kt + 1) * 128],
                                rhs=qtn[:, h0:h0 + 2, 32 * kt:512],
                                start=True, stop=True)
                        pend_exp.append(
                            lambda h0=h0, g=g, grp=grp, used=M_USED[idx],
                            sc=sc, dens=dens, cxs=cxs:
                            post_exp_m(h0, g, grp, used, sc, dens, cxs))
                pendH.append(lambda h=h0, den=dens[0], cx=cxs[0]:
                             post_head(h, den, cx))
                pendH.append(lambda h=h0 + 1, den=dens[1], cx=cxs[1]:
                             post_head(h, den, cx))
            while pend_exp or pend_acc:
                if pend_acc:
                    pend_acc.pop(0)()
                if pend_exp:
                    pend_exp.pop(0)()
            while pendH:
                pendH.pop(0)()

            # ------------------------ phase 4: out proj -------------------
            pend4 = []
            for c2 in range(ET):
                while len(pend4) > 1:
                    pend4.pop(0)()
                wo_sb = wo_tiles[c2]
                acc = pcx.tile([128, QPC], F32, tag="cx", name="oacc")
                for ct in range(ET):
                    nc.tensor.matmul(acc,
                                     lhsT=wo_sb[:, ct * 128:(ct + 1) * 128],
                                     rhs=ctxt[:, ct, :],
                                     start=(ct == 0), stop=(ct == ET - 1))

                def post_o(c2=c2, acc=acc):
                    ot = osb.tile([128, QPC], F32, tag="ot", name="ot")
                    nc.vector.tensor_scalar(
                        out=ot, in0=acc, scalar1=bo_sb[:, c2:c2 + 1],
                        scalar2=None, op0=ADD)
                    nc.sync.dma_start(
                        out=out_d[c2 * 128:(c2 + 1) * 128, :], in_=ot)
                pend4.append(post_o)
            while pend4:
                pend4.pop(0)()
    nc.compile()
    return nc


# ---------------------------------------------------------------------------
# host-side sharding
# ---------------------------------------------------------------------------

def make_in_maps(cfg, inputs):
    B, S, E, D, G = cfg["B"], cfg["S"], cfg["E"], cfg["D"], cfg["G"]
    NH, ET, NKT, QPC, GS = derived(cfg)
    x = np.asarray(inputs["x"], np.float32)
    Wq = np.asarray(inputs["Wq"], np.float32)
    Wk = np.asarray(inputs["Wk"], np.float32)
    Wv = np.asarray(inputs["Wv"], np.float32)
    Wo = np.asarray(inputs["Wo"], np.float32)

    wqp = np.ascontiguousarray(
        Wq.reshape(ET, 128, NH, 128).transpose(1, 2, 0, 3)
        .reshape(128, NH, ET * 128).astype(BF))
    wop = np.ascontiguousarray(
        Wo.reshape(ET, 128, ET, 128).transpose(1, 2, 0, 3)
        .reshape(128, ET, ET * 128).astype(BF))
    wkp = np.ascontiguousarray(
        Wk.reshape(ET, 128, G * 128).transpose(1, 0, 2)
        .reshape(128, ET * G * 128).astype(BF))
    wvp = np.ascontiguousarray(
        Wv.reshape(ET, 128, G * 128).transpose(1, 0, 2)
        .reshape(128, ET * G * 128).astype(BF))

    shared = dict(
        WqP=wqp, WoP=wop, WkP=wkp, WvP=wvp,
        bq_t=np.ascontiguousarray(
            np.asarray(inputs["bq"], np.float32).reshape(NH, 128).T),
        bk_t=np.ascontiguousarray(
            np.asarray(inputs["bk"], np.float32).reshape(G, 128).T),
        bv_r=np.ascontiguousarray(
            np.asarray(inputs["bv"], np.float32).reshape(1, G * 128)
            .astype(BF)),
        bo_t=np.ascontiguousarray(
            np.asarray(inputs["bo"], np.float32).reshape(ET, 128).T),
        gq_r=np.ascontiguousarray(
            np.asarray(inputs["gamma_q"], np.float32).reshape(1, 128)
            .astype(BF)),
        gk_r=np.ascontiguousarray(
            np.asarray(inputs["gamma_k"], np.float32).reshape(1, 128)
            .astype(BF)),
    )
    xTb = [np.ascontiguousarray(x[b].T.astype(BF)) for b in range(B)]
    in_maps, perms = [], []
    for c in range(8):
        b, j = c // 4, c % 4
        kk = np.arange(128)[:, None]
        ii = np.arange(32)[None, :]
        dmask = (kk <= 4 * ii + j).astype(BF)
        dmask2 = np.empty((128, 64), BF)
        dmask2[:, 0::2] = dmask
        dmask2[:, 1::2] = dmask
        m = dict(shared)
        m["xT"] = xTb[b]
        m["xq"] = np.ascontiguousarray(xTb[b][:, j::4])
        m["dmask"] = np.ascontiguousarray(dmask)
        m["dmask2"] = np.ascontiguousarray(dmask2)
        in_maps.append(m)
        perms.append(j)
    return in_maps, perms


def assemble(cfg, results, perms):
    B, S, E = cfg["B"], cfg["S"], cfg["E"]
    out = np.empty((B, S, E), np.float32)
    for c in range(8):
        b, j = c // 4, perms[c]
        out[b, j::4, :] = results[c]["outT"].T
    return out


_CACHE = {}


def kernel(**inputs):
    cfg = full_cfg()
    if "nc" not in _CACHE:
        _CACHE["nc"] = build_program(cfg)
    nc = _CACHE["nc"]
    in_maps, perms = make_in_maps(cfg, inputs)
    res = run_bass_kernel_spmd(nc, in_maps, list(range(8)))
    return assemble(cfg, res.results, perms)


# revision 15
# speedup vs baseline: 1.0435x; 1.0435x over previous
"""GQA attention block (RMSNorm-QK, causal, GQA) on 8 trn2 NeuronCores — v2.

Sharding: batch over groups of 4 cores; stride-4 query interleave within a
batch. Core c handles batch c//4 and query tokens {j, j+4, ..., j+2044}
(j = c%4), so the causal structure is IDENTICAL on every core: for key tile
kt (128 keys), query columns < 32*kt are fully masked (skipped entirely),
columns [32kt, 32kt+32) are diagonal (one shared [128,32] 0/1 mask), and
the rest are fully valid. Scores / exp / denominator / AV all run on the
causally-valid suffix [32kt, 512) only — ~47% less attention work than the
full rectangle, with zero collectives and one uniform SPMD program.

All matmuls are bf16 (1 cycle/row at any free size on the PE; error budget
2e-2 >> bf16's ~1e-3). Activations are feature-major ("T layout"); V is
projected token-major directly (x-tiles stationary), so the kernel needs no
transposes at all. Partition-dim reductions (RMS sum-of-squares, softmax
denominators) and per-token broadcasts are rank-1 matmuls; reciprocals use
the fast custom-DVE op (~18 bits, ~5x faster than nc.vector.reciprocal).
Score tiles for key-tile pairs (p, 16-p) pack into a single PSUM bank so
exp runs as one activation per bank. Softmax needs no max subtraction:
RMS-normalized q,k bound |scores|/sqrt(D) <= sqrt(D).
"""

import math
import numpy as np
from contextlib import ExitStack

import ml_dtypes
import concourse.bass as bass
import concourse.mybir as mybir
import concourse.tile as tile
from concourse import bacc
from concourse.bass_utils import run_bass_kernel_spmd

F32 = mybir.dt.float32
F32R = mybir.dt.float32r
BF16 = mybir.dt.bfloat16
ADD = mybir.AluOpType.add
MULT = mybir.AluOpType.mult
EXP = mybir.ActivationFunctionType.Exp
SQRT = mybir.ActivationFunctionType.Sqrt
SQUARE = mybir.ActivationFunctionType.Square

BF = ml_dtypes.bfloat16
EPS = 1e-8


def full_cfg():
    return dict(B=2, S=2048, E=2048, D=128, G=2)


def derived(cfg):
    B, S, E, D, G = cfg["B"], cfg["S"], cfg["E"], cfg["D"], cfg["G"]
    NH = E // D            # 16 query heads == E blocks of 128
    ET = E // 128          # 16 contraction tiles of E
    NKT = S // 128         # 16 key tiles
    QPC = S // 4           # 512 queries per core (stride-4 stripe)
    GS = NH // G           # 8 heads per kv group
    assert D == 128 and QPC == 512
    return NH, ET, NKT, QPC, GS


# key-tile pairs pack into one PSUM bank: widths (512-32p) + 32p = 512;
# two pairs pack into one 2-bank [128,1024] mega tile so exp runs as a
# single activation per mega tile (5 ACT calls/head instead of 16).
SC_GROUPS = [
    [(0,), (1, 15)],
    [(2, 14), (3, 13)],
    [(4, 12), (5, 11)],
    [(6, 10), (7, 9)],
    [(8,)],
]


def build_program(cfg):
    B, S, E, D, G = cfg["B"], cfg["S"], cfg["E"], cfg["D"], cfg["G"]
    NH, ET, NKT, QPC, GS = derived(cfg)
    SCALE = 1.0 / math.sqrt(D)
    KC = 512
    NKC = S // KC

    nc = bacc.Bacc()
    xT_d = nc.dram_tensor("xT", [E, S], BF16, kind="ExternalInput")
    xq_d = nc.dram_tensor("xq", [E, QPC], BF16, kind="ExternalInput")
    wq_d = nc.dram_tensor("WqP", [128, NH, ET * 128], BF16, kind="ExternalInput")
    wo_d = nc.dram_tensor("WoP", [128, ET, ET * 128], BF16, kind="ExternalInput")
    wk_d = nc.dram_tensor("WkP", [128, ET * G * 128], BF16, kind="ExternalInput")
    wv_d = nc.dram_tensor("WvP", [128, ET * G * 128], BF16, kind="ExternalInput")
    bq_d = nc.dram_tensor("bq_t", [128, NH], F32, kind="ExternalInput")
    bk_d = nc.dram_tensor("bk_t", [128, G], F32, kind="ExternalInput")
    bv_d = nc.dram_tensor("bv_r", [1, G * 128], BF16, kind="ExternalInput")
    bo_d = nc.dram_tensor("bo_t", [128, ET], F32, kind="ExternalInput")
    gq_d = nc.dram_tensor("gq_r", [1, 128], BF16, kind="ExternalInput")
    gk_d = nc.dram_tensor("gk_r", [1, 128], BF16, kind="ExternalInput")
    dm_d = nc.dram_tensor("dmask", [128, 32], BF16, kind="ExternalInput")
    out_d = nc.dram_tensor("outT", [E, QPC], F32, kind="ExternalOutput")

    def r(ap):
        return ap if ap.dtype == F32R else ap.bitcast(F32R)

    xT_r = xT_d.rearrange("(t p) s -> p t s", p=128)    # [128, ET, S]
    xq_r = xq_d.rearrange("(t p) q -> p t q", p=128)    # [128, ET, QPC]

    with tile.TileContext(nc) as tc, ExitStack() as top:
        consts = top.enter_context(tc.tile_pool(name="consts", bufs=1))
        persist = top.enter_context(tc.tile_pool(name="persist", bufs=1))
        xqp = top.enter_context(tc.tile_pool(name="xqp", bufs=1))
        wqp = top.enter_context(tc.tile_pool(name="wqs", bufs=6))

        ktn = [persist.tile([128, S], BF16, tag=f"ktn{g}", name=f"ktn{g}")
               for g in range(G)]
        vtok = persist.tile([128, NKT, G * 128], BF16, tag="vtok")
        qtn = persist.tile([128, NH, QPC], BF16, tag="qtn")
        ctxt = persist.tile([128, ET, QPC], BF16, tag="ctxt")

        # ------------- phase 1: K/V projection over all tokens ------------
        with ExitStack() as p1:
            xsp = p1.enter_context(tc.tile_pool(name="xs", bufs=4))
            wkvp = p1.enter_context(tc.tile_pool(name="wkv", bufs=1))
            tmp = p1.enter_context(tc.tile_pool(name="tmp1", bufs=3))

            # startup DMA order: tiny "starter" transfers first so the
            # first matmuls are gated on ~0.4MB, then the bulk, then
            # prefetches, then consts (needed ~20us in).
            wk_sb = wkvp.tile([128, ET * G * 128], BF16, tag="wk")
            nc.sync.dma_start(out=wk_sb[:, 0:512], in_=wk_d[:, 0:512])
            xts = []
            xt0 = xsp.tile([128, ET, KC], BF16, tag="xt", name="xt0")
            nc.sync.dma_start(out=xt0[:, 0:2, :], in_=xT_r[:, 0:2, 0:KC])
            nc.sync.dma_start(out=wk_sb[:, 512:2048], in_=wk_d[:, 512:2048])
            nc.sync.dma_start(out=xt0[:, 2:8, :], in_=xT_r[:, 2:8, 0:KC])
            nc.sync.dma_start(out=wk_sb[:, 2048:4096],
                              in_=wk_d[:, 2048:4096])
            nc.sync.dma_start(out=xt0[:, 8:16, :], in_=xT_r[:, 8:16, 0:KC])
            xts.append(xt0)
            wv_sb = wkvp.tile([128, ET * G * 128], BF16, tag="wv")
            for i in range(2):
                nc.sync.dma_start(out=wv_sb[:, i * 2048:(i + 1) * 2048],
                                  in_=wv_d[:, i * 2048:(i + 1) * 2048])
            xt1 = xsp.tile([128, ET, KC], BF16, tag="xt", name="xt1")
            nc.sync.dma_start(out=xt1, in_=xT_r[:, :, KC:2 * KC])
            xts.append(xt1)
            xq_sb = xqp.tile([128, ET, QPC], BF16, tag="xq")
            nc.sync.dma_start(out=xq_sb, in_=xq_r)

            ones_col_bf = consts.tile([128, 1], BF16)
            nc.vector.memset(ones_col_bf, 1.0)
            ones_row_bf = consts.tile([1, 128], BF16)
            nc.vector.memset(ones_row_bf, 1.0)
            eps_t = consts.tile([1, 1], F32)
            nc.vector.memset(eps_t, EPS)
            gq_sb = consts.tile([1, 128], BF16)
            nc.sync.dma_start(out=gq_sb, in_=gq_d[:, :])
            gk_sb = consts.tile([1, 128], BF16)
            nc.sync.dma_start(out=gk_sb, in_=gk_d[:, :])
            bq_sb = consts.tile([128, NH], F32)
            nc.sync.dma_start(out=bq_sb, in_=bq_d[:, :])
            bk_sb = consts.tile([128, G], F32)
            nc.sync.dma_start(out=bk_sb, in_=bk_d[:, :])
            bv_sb = consts.tile([1, G * 128], BF16)
            nc.sync.dma_start(out=bv_sb, in_=bv_d[:, :])
            bo_sb = consts.tile([128, ET], F32)
            nc.sync.dma_start(out=bo_sb, in_=bo_d[:, :])
            dmask = consts.tile([128, 32], BF16)
            nc.sync.dma_start(out=dmask, in_=dm_d[:, :])
            wq_tiles = []
            for qc in range(2):
                wq_sb = wqp.tile([128, ET * 128], BF16, tag="wq", name="wq")
                nc.sync.dma_start(out=wq_sb, in_=wq_d[:, qc, :])
                wq_tiles.append(wq_sb)
            pk = p1.enter_context(tc.tile_pool(name="pk", bufs=1, space="PSUM"))
            pv = p1.enter_context(tc.tile_pool(name="pv", bufs=1, space="PSUM"))
            pssq = p1.enter_context(tc.tile_pool(name="pssq", bufs=2, space="PSUM"))
            pbc = p1.enter_context(tc.tile_pool(name="pbc", bufs=2, space="PSUM"))

            q_top, q_mid, q_bot = [], [], []

            def pop(q):
                if q:
                    q.pop(0)()

            for kc in range(NKC):
                if kc + 2 < NKC:
                    xt = xsp.tile([128, ET, KC], BF16, tag="xt", name="xt")
                    nc.sync.dma_start(
                        out=xt, in_=xT_r[:, :, (kc + 2) * KC:(kc + 3) * KC])
                    xts.append(xt)
                xc = xts[kc]
                pop(q_top)
                acck = pk.tile([128, G, KC], F32, tag="acck", name="acck")
                for g in range(G):
                    for et in range(ET):
                        nc.tensor.matmul(
                            acck[:, g, :],
                            lhsT=wk_sb[:, et * 256 + g * 128:
                                       et * 256 + (g + 1) * 128],
                            rhs=xc[:, et, :], start=(et == 0),
                            stop=(et == ET - 1))
                pop(q_mid)
                accv = pv.tile([128, 4, G * 128], F32, tag="accv", name="accv")
                for s in range(4):
                    for et in range(ET):
                        nc.tensor.matmul(
                            accv[:, s, :],
                            lhsT=xc[:, et, s * 128:(s + 1) * 128],
                            rhs=wv_sb[:, et * 256:(et + 1) * 256],
                            start=(et == 0), stop=False)
                    nc.tensor.matmul(accv[:, s, :], lhsT=ones_row_bf,
                                     rhs=bv_sb, start=False, stop=True)
                pop(q_bot)

                def top_f(kc=kc, acck=acck, accv=accv):
                    outs = []
                    for g in range(G):
                        sq = tmp.tile([128, KC], BF16, tag="sq", name="sq",
                                      bufs=3)
                        nc.scalar.activation(out=sq, in_=acck[:, g, :],
                                             func=SQUARE,
                                             bias=bk_sb[:, g:g + 1])
                        vb = tmp.tile([128, KC], F32, tag="vb", name="vb",
                                      bufs=5)
                        nc.vector.tensor_scalar(
                            out=vb, in0=acck[:, g, :],
                            scalar1=bk_sb[:, g:g + 1], scalar2=None, op0=ADD)
                        outs.append((sq, vb))
                    for s in range(4):
                        nc.scalar.copy(out=vtok[:, kc * 4 + s, :],
                                       in_=accv[:, s, :])
                    q_mid.append(lambda kc=kc, outs=outs: mid_f(kc, outs))

                def mid_f(kc, outs):
                    outs2 = []
                    for g in range(G):
                        sq, vb = outs[g]
                        ssq = pssq.tile([1, KC], F32, tag="ssq", name="ssq")
                        nc.tensor.matmul(ssq, lhsT=ones_col_bf, rhs=sq,
                                         start=True, stop=True)
                        rms = tmp.tile([1, KC], F32, tag="rms", name="rms",
                                       bufs=3)
                        nc.scalar.activation(out=rms, in_=ssq, func=SQRT,
                                             scale=1.0 / D, bias=eps_t[:, :])
                        rinv = tmp.tile([1, KC], F32, tag="rinv", name="rinv",
                                        bufs=3)
                        nc.vector.reciprocal_approx_fast(out=rinv, in_=rms)
                        rinv_r = tmp.tile([1, KC], BF16, tag="rinvr",
                                          name="rinvr", bufs=5)
                        nc.vector.tensor_copy(out=rinv_r, in_=rinv)
                        outs2.append((vb, rinv_r))
                    q_bot.append(lambda kc=kc, outs2=outs2: bot_f(kc, outs2))

                def bot_f(kc, outs2):
                    for g in range(G):
                        vb, rinv_r = outs2[g]
                        bc = pbc.tile([128, KC], F32, tag="bc", name="bc")
                        nc.tensor.matmul(bc, lhsT=gk_sb, rhs=rinv_r,
                                         start=True, stop=True)
                        nc.vector.tensor_tensor(
                            out=ktn[g][:, kc * KC:(kc + 1) * KC],
                            in0=vb, in1=bc, op=MULT)

                q_top.append(top_f)
            while q_top or q_mid or q_bot:
                pop(q_top)
                pop(q_mid)
                pop(q_bot)

        # ------------- phase 2: Q projection (own 512 queries) ------------
        with ExitStack() as p2:
            tmp2 = p2.enter_context(tc.tile_pool(name="tmp2", bufs=3))
            pq = p2.enter_context(tc.tile_pool(name="pq", bufs=2, space="PSUM"))
            pssq2 = p2.enter_context(tc.tile_pool(name="pssq2", bufs=2,
                                                  space="PSUM"))
            pbcq = p2.enter_context(tc.tile_pool(name="pbcq", bufs=2,
                                                 space="PSUM"))
            for qc in range(2, 4):
                wq_sb = wqp.tile([128, ET * 128], BF16, tag="wq", name="wq")
                nc.sync.dma_start(out=wq_sb, in_=wq_d[:, qc, :])
                wq_tiles.append(wq_sb)
            q_top, q_mid, q_bot = [], [], []
            for qc in range(NH):
                if qc + 4 < NH:
                    wq_sb = wqp.tile([128, ET * 128], BF16, tag="wq",
                                     name="wq")
                    nc.sync.dma_start(out=wq_sb, in_=wq_d[:, qc + 4, :])
                    wq_tiles.append(wq_sb)
                pop(q_top)
                acc = pq.tile([128, QPC], F32, tag="qacc", name="qacc")
                for et in range(ET):
                    nc.tensor.matmul(
                        acc,
                        lhsT=wq_tiles[qc][:, et * 128:(et + 1) * 128],
                        rhs=xq_sb[:, et, :],
                        start=(et == 0), stop=(et == ET - 1))
                pop(q_mid)
                pop(q_bot)

                def top_f(qc=qc, acc=acc):
                    sq = tmp2.tile([128, QPC], BF16, tag="sq", name="qsq")
                    nc.scalar.activation(out=sq, in_=acc, func=SQUARE,
                                         bias=bq_sb[:, qc:qc + 1])
                    vb = tmp2.tile([128, QPC], F32, tag="vb", name="qvb",
                                   bufs=4)
                    nc.vector.tensor_scalar(
                        out=vb, in0=acc, scalar1=bq_sb[:, qc:qc + 1],
                        scalar2=None, op0=ADD)

                    def mid_f(qc=qc, sq=sq, vb=vb):
                        ssq = pssq2.tile([1, QPC], F32, tag="ssq",
                                         name="qssq")
                        nc.tensor.matmul(ssq, lhsT=ones_col_bf, rhs=sq,
                                         start=True, stop=True)
                        rms = tmp2.tile([1, QPC], F32, tag="rms", name="qrms")
                        nc.scalar.activation(out=rms, in_=ssq, func=SQRT,
                                             scale=1.0 / D, bias=eps_t[:, :])
                        rinv = tmp2.tile([1, QPC], F32, tag="rinv",
                                         name="qrinv", bufs=3)
                        nc.vector.reciprocal_approx_fast(out=rinv, in_=rms)
                        rinv_r = tmp2.tile([1, QPC], BF16, tag="rinvr",
                                           name="qrinvr", bufs=4)
                        nc.vector.tensor_copy(out=rinv_r, in_=rinv)

                        def bot_f(qc=qc, vb=vb, rinv_r=rinv_r):
                            bc = pbcq.tile([128, QPC], F32, tag="bc",
                                           name="qbc")
                            nc.tensor.matmul(bc, lhsT=gq_sb, rhs=rinv_r,
                                             start=True, stop=True)
                            nc.vector.tensor_tensor(out=qtn[:, qc, :],
                                                    in0=vb, in1=bc, op=MULT)
                        q_bot.append(bot_f)
                    q_mid.append(mid_f)
                q_top.append(top_f)
            while q_top or q_mid or q_bot:
                pop(q_top)
                pop(q_mid)
                pop(q_bot)

        # ---------- phase 3: causal attention + phase 4: out proj ---------
        with ExitStack() as p34:
            ptp = p34.enter_context(tc.tile_pool(name="pt", bufs=5))
            tmp3 = p34.enter_context(tc.tile_pool(name="tmp3", bufs=3))
            wop = p34.enter_context(tc.tile_pool(name="wos", bufs=16))
            osb = p34.enter_context(tc.tile_pool(name="osb", bufs=3))
            psc = p34.enter_context(tc.tile_pool(name="psc", bufs=2,
                                                 space="PSUM"))
            pden = p34.enter_context(tc.tile_pool(name="pden", bufs=2,
                                                  space="PSUM"))
            pcx = p34.enter_context(tc.tile_pool(name="pcx", bufs=2,
                                                 space="PSUM"))
            pend_exp = []
            pend_acc = []
            pendH = []

            def grp_offsets(grp):
                """[(kt, col offset in mega tile, width)] for a group."""
                out = []
                for pi, pair in enumerate(grp):
                    base = 512 * pi
                    for m, kt in enumerate(pair):
                        off = base + (0 if m == 0 else 512 - 32 * pair[0])
                        out.append((kt, off, 512 - 32 * kt))
                return out

            def post_exp(h, g, grp, sc, den, cx):
                used = 256 if len(grp) == 1 else 1024
                pt = ptp.tile([128, 2 * QPC], BF16, tag="pt", name="pt")
                nc.scalar.activation(out=pt[:, 0:used], in_=sc[:, 0:used],
                                     func=EXP, scale=SCALE)
                mem = grp_offsets(grp)
                for kt, off, w in mem:
                    nc.vector.tensor_tensor(out=pt[:, off:off + 32],
                                            in0=pt[:, off:off + 32],
                                            in1=dmask, op=MULT)

                def post_acc():
                    for kt, off, w in mem:
                        first = (kt == 0)
                        last = (kt == 8)
                        nc.tensor.matmul(den[:, 32 * kt:512],
                                         lhsT=ones_col_bf,
                                         rhs=pt[:, off:off + w],
                                         start=first, stop=last)
                        nc.tensor.matmul(
                            cx[:, 32 * kt:512],
                            lhsT=vtok[:, kt, g * 128:(g + 1) * 128],
                            rhs=pt[:, off:off + w],
                            start=first, stop=last)
                pend_acc.append(post_acc)

            def post_head(h, den, cx):
                rd = tmp3.tile([1, QPC], F32, tag="rd", name="rd")
                nc.vector.reciprocal_approx_fast(out=rd, in_=den)
                rd_bf = tmp3.tile([1, QPC], BF16, tag="rdbf", name="rdbf")
                nc.vector.tensor_copy(out=rd_bf, in_=rd)
                bc2 = psc.tile([128, 2 * QPC], F32, tag="sc", name="bc2")
                nc.tensor.matmul(bc2[:, 0:QPC], lhsT=ones_row_bf, rhs=rd_bf,
                                 start=True, stop=True)
                bc2s = tmp3.tile([128, QPC], F32, tag="bc2s", name="bc2s")
                nc.scalar.copy(out=bc2s, in_=bc2[:, 0:QPC])
                nc.vector.tensor_tensor(out=ctxt[:, h, :], in0=cx, in1=bc2s,
                                        op=MULT)

            # prefetch all Wo tiles while Sync is otherwise idle in phase 3
            wo_tiles = []
            for c2 in range(ET):
                wo_sb = wop.tile([128, ET * 128], BF16, tag="wo", name="wo")
                nc.sync.dma_start(out=wo_sb, in_=wo_d[:, c2, :])
                wo_tiles.append(wo_sb)

            for h in range(NH):
                g = h // GS
                den = pden.tile([1, QPC], F32, tag="den", name="den")
                cx = pcx.tile([128, QPC], F32, tag="cx", name="cx")
                for ti, grp in enumerate(SC_GROUPS):
                    if pend_acc:
                        pend_acc.pop(0)()
                    if pend_exp:
                        pend_exp.pop(0)()
                    if ti == 2 and pendH:
                        pendH.pop(0)()
                    sc = psc.tile([128, 2 * QPC], F32, tag="sc", name="sc")
                    for kt, off, w in grp_offsets(grp):
                        nc.tensor.matmul(
                            sc[:, off:off + w],
                            lhsT=ktn[g][:, kt * 128:(kt + 1) * 128],
                            rhs=qtn[:, h, 32 * kt:512],
                            start=True, stop=True)
                    pend_exp.append(
                        lambda h=h, g=g, grp=grp, sc=sc, den=den, cx=cx:
                        post_exp(h, g, grp, sc, den, cx))
                pendH.append(lambda h=h, den=den, cx=cx: post_head(h, den, cx))
            while pend_exp or pend_acc:
                if pend_acc:
                    pend_acc.pop(0)()
                if pend_exp:
                    pend_exp.pop(0)()
            while pendH:
                pendH.pop(0)()

            # ------------------------ phase 4: out proj -------------------
            pend4 = []
            for c2 in range(ET):
                while len(pend4) > 1:
                    pend4.pop(0)()
                wo_sb = wo_tiles[c2]
                acc = pcx.tile([128, QPC], F32, tag="cx", name="oacc")
                for ct in range(ET):
                    nc.tensor.matmul(acc,
                                     lhsT=wo_sb[:, ct * 128:(ct + 1) * 128],
                                     rhs=ctxt[:, ct, :],
                                     start=(ct == 0), stop=(ct == ET - 1))

                def post_o(c2=c2, acc=acc):
                    ot = osb.tile([128, QPC], F32, tag="ot", name="ot")
                    nc.vector.tensor_scalar(
                        out=ot, in0=acc, scalar1=bo_sb[:, c2:c2 + 1],
                        scalar2=None, op0=ADD)
                    nc.sync.dma_start(
                        out=out_d[c2 * 128:(c2 + 1) * 128, :], in_=ot)
                pend4.append(post_o)
            while pend4:
                pend4.pop(0)()
    nc.compile()
    return nc


# ---------------------------------------------------------------------------
# host-side sharding
# ---------------------------------------------------------------------------

def make_in_maps(cfg, inputs):
    B, S, E, D, G = cfg["B"], cfg["S"], cfg["E"], cfg["D"], cfg["G"]
    NH, ET, NKT, QPC, GS = derived(cfg)
    x = np.asarray(inputs["x"], np.float32)
    Wq = np.asarray(inputs["Wq"], np.float32)
    Wk = np.asarray(inputs["Wk"], np.float32)
    Wv = np.asarray(inputs["Wv"], np.float32)
    Wo = np.asarray(inputs["Wo"], np.float32)

    wqp = np.ascontiguousarray(
        Wq.reshape(ET, 128, NH, 128).transpose(1, 2, 0, 3)
        .reshape(128, NH, ET * 128).astype(BF))
    wop = np.ascontiguousarray(
        Wo.reshape(ET, 128, ET, 128).transpose(1, 2, 0, 3)
        .reshape(128, ET, ET * 128).astype(BF))
    wkp = np.ascontiguousarray(
        Wk.reshape(ET, 128, G * 128).transpose(1, 0, 2)
        .reshape(128, ET * G * 128).astype(BF))
    wvp = np.ascontiguousarray(
        Wv.reshape(ET, 128, G * 128).transpose(1, 0, 2)
        .reshape(128, ET * G * 128).astype(BF))

    shared = dict(
        WqP=wqp, WoP=wop, WkP=wkp, WvP=wvp,
        bq_t=np.ascontiguousarray(
            np.asarray(inputs["bq"], np.float32).reshape(NH, 128).T),
        bk_t=np.ascontiguousarray(
            np.asarray(inputs["bk"], np.float32).reshape(G, 128).T),
        bv_r=np.ascontiguousarray(
            np.asarray(inputs["bv"], np.float32).reshape(1, G * 128)
            .astype(BF)),
        bo_t=np.ascontiguousarray(
            np.asarray(inputs["bo"], np.float32).reshape(ET, 128).T),
        gq_r=np.ascontiguousarray(
            np.asarray(inputs["gamma_q"], np.float32).reshape(1, 128)
            .astype(BF)),
        gk_r=np.ascontiguousarray(
            np.asarray(inputs["gamma_k"], np.float32).reshape(1, 128)
            .astype(BF)),
    )
    xTb = [np.ascontiguousarray(x[b].T.astype(BF)) for b in range(B)]
    in_maps, perms = [], []
    for c in range(8):
        b, j = c // 4, c % 4
        kk = np.arange(128)[:, None]
        ii = np.arange(32)[None, :]
        dmask = (kk <= 4 * ii + j).astype(BF)
        m = dict(shared)
        m["xT"] = xTb[b]
        m["xq"] = np.ascontiguousarray(xTb[b][:, j::4])
        m["dmask"] = np.ascontiguousarray(dmask)
        in_maps.append(m)
        perms.append(j)
    return in_maps, perms


def assemble(cfg, results, perms):
    B, S, E = cfg["B"], cfg["S"], cfg["E"]
    out = np.empty((B, S, E), np.float32)
    for c in range(8):
        b, j = c // 4, perms[c]
        out[b, j::4, :] = results[c]["outT"].T
    return out


_CACHE = {}


def kernel(**inputs):
    cfg = full_cfg()
    if "nc" not in _CACHE:
        _CACHE["nc"] = build_program(cfg)
    nc = _CACHE["nc"]
    in_maps, perms = make_in_maps(cfg, inputs)
    res = run_bass_kernel_spmd(nc, in_maps, list(range(8)))
    return assemble(cfg, res.results, perms)
